# revision 3
# baseline (speedup 1.0000x reference)
# Fused conv3x3(same) + bias + tanh + x2 + stride-4 subsample, data-parallel
# over 8 NeuronCores.
#
# Math: out[b,oc,y,x] = 2*tanh(sum_{ic,ky,kx} w[oc,ic,ky,kx]*x[b,ic,4y+ky-1,4x+kx-1] + bias[oc])
# computed in fp16 like the reference. Since the spatial stride (4) exceeds the
# kernel size (3), every output pixel reads a disjoint 3x3x8 input patch, so the
# conv lowers exactly to a [72 -> 64] GEMM over 64*64 pixels per image. The host
# does the im2col rearrangement (pure data movement, fp16 cast is identical to
# the reference's .astype(float16)); each core runs the GEMM + bias + tanh for
# 4 of the 32 images. The trailing *2 and fp32 cast are exact in either order,
# so they are applied on the host after the fp16 tanh.
#
# Device kernel is hand-scheduled raw bacc. Pipeline: 8 half-image stages
# (2048 px each). Per stage: input DMA [72, 4096B-runs] -> 4 matmuls
# ([72->64] x 512 px, two-deep in PSUM partitions: chunk 2q+t -> partitions
# t*64.. of bank (2s+q)%8) -> one 128-partition ACT (tanh, per-partition bias
# via the ACT bias operand - no bias row in the contraction) -> output DMA.
#
# Scheduling notes (from perfetto traces):
# - HBM *reads* cap at ~16 GB/s per SDMA engine regardless of descriptor size
#   (latency-bound); *writes* at 2KB descriptors hit ~24 GB/s. Input stream
#   (~2.36 MB/core) is the long pole, so it is issued first and split across
#   BOTH HWDGE rings (sync: even stages, scalar: odd stages) for deeper
#   per-engine pipelining.
# - The contraction is exactly 72 rows (no padding): bias rides in the ACT
#   instruction's per-partition bias operand, saving 10% input bytes.
# - Scalar ACT chain (tanh, (N+352)/1.2 ns) is the serial tail; a dummy
#   1-col ACT at the scalar queue head hoists the ~1.3us ACT_TABLE_LOAD into
#   the (fixed ~7us) framework preamble.
# - The PE clock gate opens only after ~5us of sustained matmul activity
#   (cold MMs run at 1.2GHz, warm 2.4GHz): a short warmup burst bridges
#   preamble -> first real matmul, fillers bridge later input waits.
import sys

import numpy as np

try:
    import concourse.bass as bass  # noqa: F401
except ImportError:
    sys.path.insert(0, "/opt/trn_rl_repo")

import concourse.bass as bass  # noqa: F401
import concourse.bacc as bacc
import concourse.mybir as mybir
from concourse.bass_utils import run_bass_kernel_spmd

N_CORES = 8
B_FULL = 32
B_CORE = B_FULL // N_CORES  # 4 images per core
C_IN = 8
KH = KW = 3
K = C_IN * KH * KW  # 72 contraction (exact, no padding)
OC = 64
OH = OW = 64
NPIX = OH * OW  # 4096
HALF = NPIX // 2  # 2048
NH = 2 * B_CORE  # 8 half-image pipeline stages
N_WARM = 16
F16 = mybir.dt.float16
F32 = mybir.dt.float32

_PROGRAM = None


def build_program():
    from contextlib import ExitStack

    nc = bacc.Bacc("TRN2")
    xp = nc.dram_tensor("xp", [NH, K, HALF], F16, kind="ExternalInput")
    w = nc.dram_tensor("w", [K, OC], F16, kind="ExternalInput")
    b = nc.dram_tensor("b", [2 * OC, 1], F32, kind="ExternalInput")
    y = nc.dram_tensor("y", [NH, 2 * OC, HALF // 2], F16, kind="ExternalOutput")

    with ExitStack() as stack:
        w_tile = stack.enter_context(nc.sbuf_tensor([K, OC], F16))
        bias_tile = stack.enter_context(nc.sbuf_tensor([2 * OC, 1], F32))
        scratch = stack.enter_context(nc.sbuf_tensor([1, 2], F16))
        # one buffer per half-image stage -> no buffer-reuse waits
        x_bufs = stack.enter_context(nc.sbuf_tensor([K, NH, HALF], F16))
        a_bufs = stack.enter_context(nc.sbuf_tensor([2 * OC, NH, HALF // 2], F16))
        warm = stack.enter_context(nc.sbuf_tensor([2 * OC, 2 * OC], F16))
        # 8 banks of [128, 512]; stage s accumulates into banks 2s%8, 2s%8+1
        ps = stack.enter_context(nc.psum_tensor([2 * OC, 8, 512], F32))
        # Per-stage input semaphores: concurrent DMAs complete out of order,
        # so one counting sem can't tell which transfer landed.
        sx = [stack.enter_context(nc.semaphore(f"s_x{i}")) for i in range(NH)]
        s_w = stack.enter_context(nc.semaphore("s_w"))
        s_b = stack.enter_context(nc.semaphore("s_b"))
        s_warm = stack.enter_context(nc.semaphore("s_warm"))
        s_mm = stack.enter_context(nc.semaphore("s_mm"))
        s_act = stack.enter_context(nc.semaphore("s_act"))
        s_y = stack.enter_context(nc.semaphore("s_y"))
        block = stack.enter_context(nc.Block())

        @block.gpsimd
        def _(gpsimd):
            gpsimd.memset(warm[:], 0.0).then_inc(s_warm, 1)

        @block.sync
        def _(sync):
            # stage 0 heads the critical path; w/bias are tiny. Even stages
            # here, odd stages on the scalar HWDGE ring (two physical rings
            # -> deeper per-engine read pipelines).
            sync.dma_start(out=x_bufs[:, 0, :], in_=xp[0]).then_inc(sx[0], 16)
            sync.dma_start(out=w_tile[:], in_=w[:]).then_inc(s_w, 16)
            sync.dma_start(out=bias_tile[:], in_=b[:]).then_inc(s_b, 16)
            for i in range(2, NH, 2):
                sync.dma_start(out=x_bufs[:, i, :], in_=xp[i]).then_inc(sx[i], 16)
            # output stores, paced by the ACT chain
            for i in range(NH):
                sync.wait_ge(s_act, i + 1)
                sync.dma_start(out=y[i], in_=a_bufs[:, i]).then_inc(s_y, 16)
            sync.wait_ge(s_y, 16 * NH)

        @block.scalar
        def _(scalar):
            # dummy 1-col activation: hoists the ACT_TABLE_LOAD to the queue
            # head so it overlaps the framework preamble instead of delaying
            # the first real ACT. Reads/writes a scratch tile (garbage ok).
            nc.scalar.activation(
                scratch[:, 0:1], scratch[:, 1:2], mybir.ActivationFunctionType.Tanh
            )
            for i in range(1, NH, 2):
                scalar.dma_start(out=x_bufs[:, i, :], in_=xp[i]).then_inc(sx[i], 16)
            for i in range(NH):
                scalar.wait_ge(s_mm, i + 1)
                if i == 0:
                    scalar.wait_ge(s_b, 16)
                bk = (2 * i) % 8
                nc.scalar.activation(
                    a_bufs[:, i],
                    ps[:, bk : bk + 2, :].rearrange("p b c -> p (b c)"),
                    mybir.ActivationFunctionType.Tanh,
                    bias=bias_tile[:, 0:1],
                ).then_inc(s_act, 1)

        @block.tensor
        def _(tensor):
            # keep the PE busy from preamble-exit until stage-0 input lands so
            # the HAM clock gate ramp starts early; results land in bank 7
            # which stage 3 later overwrites (start=True)
            tensor.wait_ge(s_warm, 1)
            for _ in range(N_WARM):
                nc.tensor.matmul(
                    ps[:OC, 7, :128],
                    warm[:, :OC],
                    warm[:],
                    start=True,
                    stop=True,
                )
            for i in range(NH):
                if i == 0:
                    tensor.wait_ge(s_w, 16)
                if i >= 4:
                    # psum bank pair reused; wait until ACT of stage i-4 read
                    # it. Taken BEFORE the input wait so the fillers below may
                    # touch this stage's banks.
                    tensor.wait_ge(s_act, i - 3)
                    # fillers: keep the PE busy across the input wait so the
                    # clock gate stays open; they write this stage's own
                    # first bank, which the real start=True matmuls overwrite
                    for _ in range(2):
                        nc.tensor.matmul(
                            ps[:OC, (2 * i) % 8, :128],
                            warm[:, :OC],
                            warm[:],
                            start=True,
                            stop=True,
                        )
                tensor.wait_ge(sx[i], 16)
                last = None
                for t in range(2):
                    for q in range(2):
                        c = 2 * q + t  # chunk within this half-image
                        last = nc.tensor.matmul(
                            ps[t * OC : (t + 1) * OC, (2 * i + q) % 8, :],
                            w_tile[:],
                            x_bufs[:, i, c * 512 : (c + 1) * 512],
                            start=True,
                            stop=True,
                        )
                last.then_inc(s_mm, 1)

    nc.finalize()
    return nc


def _get_program():
    global _PROGRAM
    if _PROGRAM is None:
        _PROGRAM = build_program()
    return _PROGRAM


def _im2col(x: np.ndarray) -> np.ndarray:
    """[B,8,256,256] fp32 -> [B,72,4096] fp16 patches, p=(ky*3+kx)*8+ic."""
    B, C, H, W = x.shape
    xh = x.astype(np.float16)
    xpad = np.zeros((B, C, H + 2, W + 2), np.float16)
    xpad[:, :, 1 : H + 1, 1 : W + 1] = xh
    s = xpad.strides
    # windows[b,c,ky,kx,y,x] = xpad[b,c,4y+ky,4x+kx] = x[b,c,4y+ky-1,4x+kx-1]
    win = np.lib.stride_tricks.as_strided(
        xpad,
        shape=(B, C, KH, KW, OH, OW),
        strides=(s[0], s[1], s[2], s[3], 4 * s[2], 4 * s[3]),
    )
    out = np.empty((B, K, NPIX), np.float16)
    np.copyto(
        out.reshape(B, KH, KW, C, OH, OW), win.transpose(0, 2, 3, 1, 4, 5)
    )
    return out


def run_sharded(x, weight, bias, **spmd_kwargs):
    """Returns (output, BassKernelResults). spmd_kwargs e.g. trace=True."""
    patches = _im2col(x)  # [32, 72, 4096] f16, contiguous
    w_mat = np.ascontiguousarray(
        weight.transpose(2, 3, 1, 0).reshape(K, OC).astype(np.float16)
    )
    b_vec = np.ascontiguousarray(
        np.tile(bias.astype(np.float32).reshape(OC), 2)[:, None]
    )

    in_maps = [
        {
            # stage s = (image s//2, half s%2): [8, 72, 2048] per core
            "xp": np.ascontiguousarray(
                patches[c * B_CORE : (c + 1) * B_CORE]
                .reshape(B_CORE, K, 2, HALF)
                .transpose(0, 2, 1, 3)
                .reshape(NH, K, HALF)
            ),
            "w": w_mat,
            "b": b_vec,
        }
        for c in range(N_CORES)
    ]
    nc = _get_program()
    res = run_bass_kernel_spmd(nc, in_maps, list(range(N_CORES)), **spmd_kwargs)
    # y core shard: [8 half-stages, 128, 1024]; stage i = (image i//2, half
    # i%2); partition p = t*64+oc; column = q*512+col; pixel chunk = 4h+2q+t
    y16 = np.concatenate([r["y"] for r in res.results], axis=0)  # [64,128,1024]
    y16 = (
        y16.reshape(B_FULL, 2, 2, OC, 2, 512)  # [b, h, t, oc, q, col]
        .transpose(0, 3, 1, 4, 2, 5)  # [b, oc, h, q, t, col]
        .reshape(B_FULL, OC, NPIX)
    )
    # 2*tanh in fp16 then cast to fp32 == cast then *2 (exact: *2 is an
    # exponent bump, in-range for |tanh|<=1)
    out = y16.astype(np.float32).reshape(B_FULL, OC, OH, OW) * np.float32(2.0)
    return out, res


def kernel(x: np.ndarray, weight: np.ndarray, bias: np.ndarray) -> np.ndarray:
    return run_sharded(x, weight, bias)[0]
